# revision 2
# baseline (speedup 1.0000x reference)
"""Trainium2 Bass kernel for AttentionConditionGenerator — fp8 DoubleRow redesign.

Math (per row b; all biases in setup are zero but handled generally):
    s = (xc + xt)/2 ; d = xc - xt
    score = A @ d              (A folded from q, w_k; tiny: std ~0.03)
    ab    = tanh(score/2)      (= 2*(attn0 - 1/2))
    v_d   = (w_v/2) @ d
    u     = ab_bcast * v_d     (|u| ~ 0.01 -> error-tolerant)
    y     = Wos @ s + w_o @ u + bod      (Wos = w_o @ w_v folded on host)
    x     = LN1(y); h = gelu(x @ w1.T + b1); out = LN2(x + h @ w2.T + b2)

Precision strategy (validated vs reference in fp64 sim):
  - All heavy matmuls in fp8-e4m3 DoubleRow (2 k-tiles per PE pass).
  - Critical operands are SPLIT into fp8 pairs (val = Q1 + Q2, same scale,
    PSUM accumulates all passes):  Wos/s (3-pass), w1/x (3-pass), w2/h
    (3-pass). Delta path (score, v_d, w_o@u) single-pass fp8: its
    contribution is ~1% of y so fp8 noise lands ~2.5e-4.
  - LayerNorms run feature-major: column sums via ones-matmul on PE,
    mean/rstd broadcast back across partitions via K=1 matmuls.

Everything stays feature-major end to end; the host pre-transposes inputs
and post-transposes the output (no PE/DMA transposes on device).
"""

import numpy as np
import ml_dtypes

try:
    import concourse.bass as bass
except ImportError:  # pragma: no cover - path setup for fresh environments
    import sys

    for _p in ("/opt/trn_rl_repo", "/root/.axon_site/_ro/trn_rl_repo"):
        if _p not in sys.path:
            sys.path.insert(0, _p)
    import concourse.bass as bass

import concourse.mybir as mybir
import concourse.tile as tile
from concourse import bacc
from concourse.bass_utils import run_bass_kernel_spmd

F32 = mybir.dt.float32
BF16 = mybir.dt.bfloat16
FP8 = mybir.dt.float8e4
DR = mybir.MatmulPerfMode.DoubleRow
AF = mybir.ActivationFunctionType
NPBF16 = ml_dtypes.bfloat16
E4 = ml_dtypes.float8_e4m3

D = 1024
H = 16
HD = 64
FFN = 4096
EPS = 1e-5
N_CORES = 8
B_TOTAL = 16384
B_CORE = B_TOTAL // N_CORES  # 2048
KT = D // 128       # 8 k-tiles of the D contraction
FKT = FFN // 128    # 32 k-tiles of the FFN contraction
MT = D // 128       # 8 D-output m-tiles
FMT = FFN // 128    # 32 FFN-output m-tiles


def build_program(b_core, r_blk, scales, trivial_ln1, trivial_ln2):
    nb = b_core // r_blk
    N = r_blk
    inv_d = 1.0 / D

    (c_tanh, c_wos, c_w1, c_w2) = scales  # float descale constants

    nc = bacc.Bacc("TRN2", target_bir_lowering=False)

    # ---- DRAM I/O ------------------------------------------------------
    s1_d = nc.dram_tensor("s1", [128, KT, b_core], FP8, kind="ExternalInput")
    s2_d = nc.dram_tensor("s2", [128, KT, b_core], FP8, kind="ExternalInput")
    d8_d = nc.dram_tensor("d8", [128, KT, b_core], FP8, kind="ExternalInput")
    at_d = nc.dram_tensor("at8", [128, KT, H], FP8, kind="ExternalInput")
    e_d = nc.dram_tensor("eb", [H, MT, 128], BF16, kind="ExternalInput")
    wv_d = nc.dram_tensor("wv8", [128, KT, D], FP8, kind="ExternalInput")
    wo_d = nc.dram_tensor("wo8", [128, KT, D], FP8, kind="ExternalInput")
    wos1_d = nc.dram_tensor("wos1", [128, KT, D], FP8, kind="ExternalInput")
    wos2_d = nc.dram_tensor("wos2", [128, KT, D], FP8, kind="ExternalInput")
    w11_d = nc.dram_tensor("w11", [128, KT, FFN], FP8, kind="ExternalInput")
    w12_d = nc.dram_tensor("w12", [128, KT, FFN], FP8, kind="ExternalInput")
    w21_d = nc.dram_tensor("w21", [128, MT, FKT, 128], FP8, kind="ExternalInput")
    w22_d = nc.dram_tensor("w22", [128, MT, FKT, 128], FP8, kind="ExternalInput")
    bod_d = nc.dram_tensor("bod", [128, MT], F32, kind="ExternalInput")
    b1_d = nc.dram_tensor("b1p", [128, FMT], F32, kind="ExternalInput")
    b2_d = nc.dram_tensor("b2p", [128, MT], F32, kind="ExternalInput")
    if not trivial_ln1:
        g1_d = nc.dram_tensor("g1p", [128, MT], F32, kind="ExternalInput")
        c1_d = nc.dram_tensor("c1p", [128, MT], F32, kind="ExternalInput")
    if not trivial_ln2:
        g2_d = nc.dram_tensor("g2p", [128, MT], F32, kind="ExternalInput")
        c2_d = nc.dram_tensor("c2p", [128, MT], F32, kind="ExternalInput")
    out_d = nc.dram_tensor("out", [128, MT, b_core], F32, kind="ExternalOutput")

    with tile.TileContext(nc) as tc:
        with (
            tc.tile_pool(name="consts", bufs=1) as consts,
            tc.tile_pool(name="wstream", bufs=3) as wstream,
            tc.tile_pool(name="xin", bufs=2) as xin,
            tc.tile_pool(name="act", bufs=2) as act,
            tc.tile_pool(name="st", bufs=2) as st,
            tc.tile_pool(name="ps_mm", bufs=2, space="PSUM") as ps_mm,
            tc.tile_pool(name="ps_av", bufs=2, space="PSUM") as ps_av,
            tc.tile_pool(name="ps_st", bufs=1, space="PSUM") as ps_st,
        ):
            # ---- constants --------------------------------------------
            at_s = consts.tile([128, KT, H], FP8)
            nc.sync.dma_start(at_s, at_d[:, :, :])
            e_s = consts.tile([H, MT, 128], BF16)
            nc.sync.dma_start(e_s, e_d[:, :, :])
            bod_s = consts.tile([128, MT], F32)
            nc.sync.dma_start(bod_s, bod_d[:, :])
            b1_s = consts.tile([128, FMT], F32)
            nc.sync.dma_start(b1_s, b1_d[:, :])
            b2_s = consts.tile([128, MT], F32)
            nc.sync.dma_start(b2_s, b2_d[:, :])
            g1_s = c1_s = g2_s = c2_s = None
            if not trivial_ln1:
                g1_s = consts.tile([128, MT], F32)
                nc.sync.dma_start(g1_s, g1_d[:, :])
                c1_s = consts.tile([128, MT], F32)
                nc.sync.dma_start(c1_s, c1_d[:, :])
            if not trivial_ln2:
                g2_s = consts.tile([128, MT], F32)
                nc.sync.dma_start(g2_s, g2_d[:, :])
                c2_s = consts.tile([128, MT], F32)
                nc.sync.dma_start(c2_s, c2_d[:, :])
            eps_s = consts.tile([1, 1], F32)
            nc.vector.memset(eps_s, EPS)
            ctanh_s = consts.tile([16, 1], F32)
            nc.vector.memset(ctanh_s, c_tanh)
            cwos_s = consts.tile([128, 1], F32)
            nc.vector.memset(cwos_s, c_wos)
            cw1_s = consts.tile([128, 1], F32)
            nc.vector.memset(cw1_s, c_w1)
            cw2_s = consts.tile([128, 1], F32)
            nc.vector.memset(cw2_s, c_w2)
            invd_s = consts.tile([1, 1], F32)
            nc.vector.memset(invd_s, inv_d)
            ones_st = consts.tile([128, 1], BF16)
            nc.vector.memset(ones_st, 1.0)
            ones_p = consts.tile([1, 128], BF16)
            nc.vector.memset(ones_p, 1.0)
            ones_n = consts.tile([1, 128], BF16)
            nc.vector.memset(ones_n, -1.0)

            def ln_stats_begin(nw=None):
                nw = nw or N
                p1 = ps_st.tile([1, nw], F32, tag="st1")
                p2 = ps_st.tile([1, nw], F32, tag="st2")
                return p1, p2

            def ln_stats_accum(p1, p2, m, vt, v2):
                nc.tensor.matmul(p1, ones_st, vt,
                                 start=(m == 0), stop=(m == MT - 1))
                nc.tensor.matmul(p2, ones_st, v2,
                                 start=(m == 0), stop=(m == MT - 1))

            def ln_stats_finish(p1, p2, nw=None):
                nw = nw or N
                mu = st.tile([1, nw], F32, tag="mu", bufs=1)
                nc.scalar.activation(mu, p1, AF.Identity, scale=invd_s)
                mq = st.tile([1, nw], F32, tag="mq", bufs=1)
                nc.scalar.activation(mq, p2, AF.Identity, scale=invd_s)
                mu2 = st.tile([1, nw], F32, tag="mu2", bufs=1)
                nc.vector.tensor_mul(mu2, mu, mu)
                var = st.tile([1, nw], F32, tag="var", bufs=1)
                nc.vector.tensor_tensor(var, mq, mu2, op=mybir.AluOpType.subtract)
                std = st.tile([1, nw], F32, tag="std", bufs=1)
                nc.scalar.activation(std, var, AF.Sqrt, bias=eps_s)
                rs = st.tile([1, nw], BF16, tag="rs", bufs=1)
                murs = st.tile([1, nw], BF16, tag="murs", bufs=1)
                with nc.allow_low_precision(reason="LN scale rows in bf16"):
                    nc.vector.reciprocal(rs, std)
                    nc.vector.tensor_mul(murs, mu, rs)
                pA = ps_av.tile([128, nw], F32, tag="vd")
                nc.tensor.matmul(pA, ones_p, rs, start=True, stop=True)
                pB = ps_av.tile([128, nw], F32, tag="ab")
                nc.tensor.matmul(pB, ones_n, murs, start=True, stop=True)
                a_b = st.tile([128, nw], BF16, tag="a_b", name="a_b")
                nc.scalar.copy(a_b, pA)
                b_b = st.tile([128, nw], BF16, tag="b_b", name="b_b")
                nc.scalar.copy(b_b, pB)
                return a_b, b_b

            def dr_chain(p, w_ap, x_ap, nkt, start, stop):
                """Accumulate nkt k-tiles of fp8 DoubleRow matmuls into p."""
                half = nkt // 2
                for k in range(half):
                    nc.tensor.matmul(p, w_ap[:, 2 * k:2 * k + 2, :],
                                     x_ap[:, 2 * k:2 * k + 2, :],
                                     start=(start and k == 0),
                                     stop=(stop and k == half - 1),
                                     perf_mode=DR)

            # ---- main loop over row blocks ----------------------------
            def load_inputs(blk):
                r0 = blk * N
                dt = xin.tile([128, KT, N], FP8, tag="dt", name="dt")
                nc.sync.dma_start(dt, d8_d[:, :, r0:r0 + N])
                x1t = xin.tile([128, KT, N], FP8, tag="x1t", name="x1t")
                nc.sync.dma_start(x1t, s1_d[:, :, r0:r0 + N])
                x2t = xin.tile([128, KT, N], FP8, tag="x2t", name="x2t")
                nc.sync.dma_start(x2t, s2_d[:, :, r0:r0 + N])
                return x1t, x2t, dt

            def prefetch_ywts():
                tiles = []
                for mh in range(2):
                    t = wstream.tile([128, KT, 512], FP8, tag="wvo", bufs=6,
                                     name="wv_s")
                    nc.sync.dma_start(t, wv_d[:, :, 512 * mh:512 * (mh + 1)])
                    tiles.append(t)
                t = wstream.tile([128, KT, 512], FP8, tag="wvo", bufs=6,
                                 name="ws1_s")
                nc.sync.dma_start(t, wos1_d[:, :, 0:512])
                tiles.append(t)
                t = wstream.tile([128, KT, 512], FP8, tag="wvo", bufs=6,
                                 name="ws2_s")
                nc.sync.dma_start(t, wos2_d[:, :, 0:512])
                tiles.append(t)
                t = wstream.tile([128, KT, 512], FP8, tag="wvo", bufs=6,
                                 name="wo_s")
                nc.sync.dma_start(t, wo_d[:, :, 0:512])
                tiles.append(t)
                return tiles

            nxt_in = load_inputs(0)
            wpre = prefetch_ywts()
            for blk in range(nb):
                r0 = blk * N
                s1t, s2t, dt = nxt_in

                # -- scores + ab = tanh(score/2) ------------------------
                psc = ps_st.tile([16, N], F32, tag="st1")
                dr_chain(psc, at_s, dt, KT, True, True)
                ab = st.tile([16, N], BF16, tag="ab", name="ab")
                nc.scalar.activation(ab, psc, AF.Tanh, scale=ctanh_s)

                # -- y phase: per half (512 cols), stream weights -------
                y_t = []
                y2_t = []
                pS1, pS2 = ln_stats_begin()
                u8_full = act.tile([128, KT, N], FP8, tag="u8", bufs=1, name="u8")
                for mh in range(MT // 4):
                    wv_s = wpre.pop(0)
                    for mm in range(4):
                        m = 4 * mh + mm
                        cs = slice(128 * mm, 128 * (mm + 1))
                        # v_d (1-pass) then u8 = (E*c)@ab * pvd
                        pvd = ps_av.tile([128, N], F32, tag="vd")
                        dr_chain(pvd, wv_s[:, :, cs], dt, KT, True, True)
                        pab = ps_av.tile([128, N], F32, tag="ab")
                        nc.tensor.matmul(pab, e_s[:, m, :], ab,
                                         start=True, stop=True)
                        ab_b = st.tile([128, N], BF16, tag="ab_b", name="ab_b")
                        nc.scalar.copy(ab_b, pab)
                        nc.vector.tensor_mul(u8_full[:, m, :], ab_b, pvd)
                for mh in range(MT // 4):
                    if mh == 0:
                        ws1_s, ws2_s, wo_s = wpre.pop(0), wpre.pop(0), wpre.pop(0)
                    else:
                        ws1_s = wstream.tile([128, KT, 512], FP8, tag="wvo",
                                             bufs=6, name="ws1_s")
                        nc.sync.dma_start(
                            ws1_s, wos1_d[:, :, 512 * mh:512 * (mh + 1)])
                        ws2_s = wstream.tile([128, KT, 512], FP8, tag="wvo",
                                             bufs=6, name="ws2_s")
                        nc.sync.dma_start(
                            ws2_s, wos2_d[:, :, 512 * mh:512 * (mh + 1)])
                        wo_s = wstream.tile([128, KT, 512], FP8, tag="wvo",
                                            bufs=6, name="wo_s")
                        nc.sync.dma_start(
                            wo_s, wo_d[:, :, 512 * mh:512 * (mh + 1)])
                    for mm in range(4):
                        m = 4 * mh + mm
                        cs = slice(128 * mm, 128 * (mm + 1))
                        # y = Wos(3-pass)@s + w_o@u
                        py = ps_mm.tile([128, N], F32, tag="mm")
                        dr_chain(py, ws1_s[:, :, cs], s1t, KT, True, False)
                        dr_chain(py, ws2_s[:, :, cs], s1t, KT, False, False)
                        dr_chain(py, ws1_s[:, :, cs], s2t, KT, False, False)
                        dr_chain(py, wo_s[:, :, cs], u8_full, KT, False, True)
                        if m > 0:
                            ln_stats_accum(pS1, pS2, m - 1, y_t[m - 1],
                                           y2_t[m - 1])
                        yt = act.tile([128, N], BF16, tag="y", bufs=MT,
                                      name="yt")
                        nc.scalar.activation(yt, py, AF.Identity,
                                             scale=cwos_s,
                                             bias=bod_s[:, m:m + 1])
                        y2 = act.tile([128, N], BF16, tag="y2", bufs=MT,
                                      name="y2")
                        nc.scalar.activation(y2, py, AF.Square,
                                             scale=cwos_s,
                                             bias=bod_s[:, m:m + 1])
                        y_t.append(yt)
                        y2_t.append(y2)
                ln_stats_accum(pS1, pS2, MT - 1, y_t[MT - 1], y2_t[MT - 1])

                # -- LN1 -> xk (bf16), X1, X2 (fp8) ---------------------
                pA, pB = ln_stats_finish(pS1, pS2)
                xk_t = []
                x1f = act.tile([128, KT, N], FP8, tag="x1f", bufs=1, name="x1f")
                x2f = act.tile([128, KT, N], FP8, tag="x2f", bufs=1, name="x2f")
                for m in range(MT):
                    t = st.tile([128, N], BF16, tag="lt", name="lt")
                    with nc.allow_low_precision(reason="LN normalize in bf16"):
                        nc.vector.tensor_mul(t, y_t[m], pA)
                    xk = act.tile([128, N], BF16, tag="xk", bufs=MT, name="xk")
                    if trivial_ln1:
                        nc.vector.tensor_add(xk, t, pB)
                    else:
                        xg = st.tile([128, N], F32, tag="xg", name="xg")
                        nc.vector.tensor_add(xg, t, pB)
                        nc.vector.tensor_scalar(
                            xk, xg, g1_s[:, m:m + 1], c1_s[:, m:m + 1],
                            op0=mybir.AluOpType.mult,
                            op1=mybir.AluOpType.add)
                    nc.scalar.copy(x1f[:, m, :], xk)
                    if m % 2 == 0:
                        nc.gpsimd.tensor_tensor(x2f[:, m, :], xk,
                                                x1f[:, m, :],
                                                op=mybir.AluOpType.subtract)
                    else:
                        nc.vector.tensor_tensor(x2f[:, m, :], xk,
                                                x1f[:, m, :],
                                                op=mybir.AluOpType.subtract)
                    xk_t.append(xk)

                # -- FFN1: h = gelu(w1@x + b1), 3-pass ------------------
                h1f = act.tile([128, FKT, N], FP8, tag="h1f", bufs=1, name="h1f")
                h2f = act.tile([128, FKT, N], FP8, tag="h2f", bufs=1, name="h2f")
                for mh in range(FMT // 4):
                    w11_s = wstream.tile([128, KT, 512], FP8, tag="w1s", bufs=4,
                                         name="w11_s")
                    nc.sync.dma_start(w11_s,
                                      w11_d[:, :, 512 * mh:512 * (mh + 1)])
                    w12_s = wstream.tile([128, KT, 512], FP8, tag="w1s", bufs=4,
                                         name="w12_s")
                    nc.sync.dma_start(w12_s,
                                      w12_d[:, :, 512 * mh:512 * (mh + 1)])
                    for mm in range(4):
                        m = 4 * mh + mm
                        cs = slice(128 * mm, 128 * (mm + 1))
                        ph = ps_mm.tile([128, N], F32, tag="mm")
                        dr_chain(ph, w11_s[:, :, cs], x1f, KT, True, False)
                        dr_chain(ph, w12_s[:, :, cs], x1f, KT, False, False)
                        dr_chain(ph, w11_s[:, :, cs], x2f, KT, False, True)
                        hb = st.tile([128, N], BF16, tag="hb", bufs=4,
                                     name="hb")
                        nc.scalar.activation(hb, ph, AF.Gelu, scale=cw1_s,
                                             bias=b1_s[:, m:m + 1])
                        if m >= FMT - 4:
                            nc.scalar.copy(h1f[:, m, :], hb)
                        else:
                            nc.gpsimd.tensor_copy(h1f[:, m, :], hb)
                        nc.vector.tensor_tensor(h2f[:, m, :], hb,
                                                h1f[:, m, :],
                                                op=mybir.AluOpType.subtract)

                # prefetch next block's inputs
                if blk + 1 < nb:
                    nxt_in = load_inputs(blk + 1)

                # -- FFN2 + residual add + LN2 (column-ranged) ----------
                def ffn2_ln2(c0, cn, last):
                    nw = cn - c0
                    z_t = []
                    z2_t = []
                    pT1, pT2 = ln_stats_begin(nw)
                    for m in range(MT):
                        w21_s = wstream.tile([128, FKT, 128], FP8, tag="w2s",
                                             bufs=4, name="w21_s")
                        nc.sync.dma_start(w21_s, w21_d[:, m, :, :])
                        w22_s = wstream.tile([128, FKT, 128], FP8, tag="w2s",
                                             bufs=4, name="w22_s")
                        nc.sync.dma_start(w22_s, w22_d[:, m, :, :])
                        pf = ps_mm.tile([128, nw], F32, tag="mm")
                        dr_chain(pf, w21_s, h1f[:, :, c0:cn], FKT, True, False)
                        dr_chain(pf, w22_s, h1f[:, :, c0:cn], FKT, False, False)
                        dr_chain(pf, w21_s, h2f[:, :, c0:cn], FKT, False, True)
                        if m > 0:
                            ln_stats_accum(pT1, pT2, m - 1, z_t[m - 1],
                                           z2_t[m - 1])
                        ft = st.tile([128, nw], F32, tag="ft", name="ft")
                        nc.scalar.activation(ft, pf, AF.Identity, scale=cw2_s,
                                             bias=b2_s[:, m:m + 1])
                        zt = act.tile([128, nw], BF16, tag="z", bufs=MT,
                                      name="zt")
                        nc.vector.tensor_add(zt, xk_t[m][:, c0:cn], ft)
                        z2 = act.tile([128, nw], BF16, tag="zz", bufs=MT,
                                      name="z2")
                        nc.gpsimd.tensor_mul(z2, zt, zt)
                        z_t.append(zt)
                        z2_t.append(z2)
                    ln_stats_accum(pT1, pT2, MT - 1, z_t[MT - 1], z2_t[MT - 1])
                    if last and blk + 1 < nb:
                        globals()
                    pA2, pB2 = ln_stats_finish(pT1, pT2, nw)
                    for m in range(MT):
                        t2 = st.tile([128, nw], BF16, tag="lt", name="t2")
                        with nc.allow_low_precision(
                                reason="LN normalize in bf16"):
                            nc.vector.tensor_mul(t2, z_t[m], pA2)
                        ot = st.tile([128, nw], F32, tag="ot", bufs=2,
                                     name="ot")
                        if trivial_ln2:
                            if blk + 1 == nb and m % 2 == 0:
                                nc.vector.tensor_add(ot, t2, pB2)
                            else:
                                nc.gpsimd.tensor_add(ot, t2, pB2)
                        else:
                            og = st.tile([128, nw], F32, tag="og", name="og")
                            nc.gpsimd.tensor_add(og, t2, pB2)
                            nc.vector.tensor_scalar(
                                ot, og, g2_s[:, m:m + 1], c2_s[:, m:m + 1],
                                op0=mybir.AluOpType.mult,
                                op1=mybir.AluOpType.add)
                        nc.sync.dma_start(out_d[:, m, r0 + c0:r0 + cn], ot)

                if blk + 1 < nb:
                    ffn2_ln2(0, N, True)
                    wpre = prefetch_ywts()
                else:
                    ffn2_ln2(0, N, True)

    nc.compile()
    return nc


def _pow2scale(a, target=192.0):
    m = float(np.abs(a).max())
    return float(2.0 ** np.floor(np.log2(target / max(m, 1e-30))))


def _split8(a, scale):
    a1 = np.asarray(a * scale, np.float32).astype(E4)
    a2 = (np.asarray(a * scale, np.float32)
          - a1.astype(np.float32)).astype(E4)
    return a1, a2


def _fm_weight(wT):
    """(d_in, d_out) -> [128, d_in/128, d_out] stationary layout."""
    return np.ascontiguousarray(
        wT.reshape(-1, 128, wT.shape[1]).transpose(1, 0, 2))


def _fm_acts(x):
    """(B, D) -> [128, KT, B] feature-major fp8 layout (already fp8)."""
    return np.ascontiguousarray(x.T.reshape(KT, 128, -1).transpose(1, 0, 2))


def host_prepare(inputs):
    f64 = {k: np.asarray(v, np.float64) for k, v in inputs.items()}
    qs = (f64["dom_movie"] @ f64["w_q"].T + f64["b_q"]) / np.sqrt(HD)
    A = np.einsum("hd,hdD->hD", qs.reshape(H, HD),
                  f64["w_k"].reshape(H, HD, D))  # (H, D)
    Wos = f64["w_o"] @ f64["w_v"]
    bod = f64["b_o"] + f64["dom_movie"][0] + f64["w_o"] @ f64["b_v"]
    wvh = 0.5 * f64["w_v"]

    sa = _pow2scale(A)
    swv = _pow2scale(wvh)
    swo = _pow2scale(f64["w_o"])
    swos = _pow2scale(Wos)
    sw1 = _pow2scale(f64["w1"])
    sw2 = _pow2scale(f64["w2"])
    # u8 = u * su must match the Wos psum scale: su = swos (ss=1) vs swo:
    su = swos / swo
    c_e = su / swv  # fold into E (with sd = 1)

    Q1, Q2 = _split8(Wos.T, swos)
    W11, W12 = _split8(f64["w1"].T, sw1)
    W21, W22 = _split8(f64["w2"].T, sw2)

    E = np.zeros((H, MT, 128), np.float32)
    for m in range(MT):
        for p in range(128):
            E[2 * m + p // 64, m, p] = c_e

    prep = {
        "at8": _fm_weight(np.asarray(A.T * sa, np.float32).astype(E4)),
        "eb": E.astype(NPBF16),
        "wv8": _fm_weight(np.asarray(wvh.T * swv, np.float32).astype(E4)),
        "wo8": _fm_weight(np.asarray(f64["w_o"].T * swo, np.float32).astype(E4)),
        "wos1": _fm_weight(Q1),
        "wos2": _fm_weight(Q2),
        "w11": _fm_weight(W11),
        "w12": _fm_weight(W12),
        "w21": np.ascontiguousarray(
            W21.reshape(FKT, 128, MT, 128).transpose(1, 2, 0, 3)),
        "w22": np.ascontiguousarray(
            W22.reshape(FKT, 128, MT, 128).transpose(1, 2, 0, 3)),
        "bod": np.ascontiguousarray(bod.reshape(MT, 128).T).astype(np.float32),
        "b1p": np.ascontiguousarray(
            f64["b1"].reshape(FMT, 128).T).astype(np.float32),
        "b2p": np.ascontiguousarray(
            f64["b2"].reshape(MT, 128).T).astype(np.float32),
    }
    trivial_ln1 = bool(np.all(f64["ln1_g"] == 1.0) and np.all(f64["ln1_b"] == 0.0))
    trivial_ln2 = bool(np.all(f64["ln2_g"] == 1.0) and np.all(f64["ln2_b"] == 0.0))
    if not trivial_ln1:
        prep["g1p"] = np.ascontiguousarray(
            f64["ln1_g"].reshape(MT, 128).T).astype(np.float32)
        prep["c1p"] = np.ascontiguousarray(
            f64["ln1_b"].reshape(MT, 128).T).astype(np.float32)
    if not trivial_ln2:
        prep["g2p"] = np.ascontiguousarray(
            f64["ln2_g"].reshape(MT, 128).T).astype(np.float32)
        prep["c2p"] = np.ascontiguousarray(
            f64["ln2_b"].reshape(MT, 128).T).astype(np.float32)

    scales = (0.5 / sa,          # tanh(score/2) from psc (sd = 1)
              1.0 / swos,        # y drain (ss = 1)
              1.0 / sw1,         # pre-gelu drain (sx = 1)
              1.0 / sw2)         # ffn2 drain (sh = 1)
    return prep, scales, trivial_ln1, trivial_ln2


_PROGRAM_CACHE = {}


def _get_program(b_core, r_blk, scales, t1, t2):
    key = (b_core, r_blk, scales, t1, t2)
    if key not in _PROGRAM_CACHE:
        _PROGRAM_CACHE[key] = build_program(b_core, r_blk, scales, t1, t2)
    return _PROGRAM_CACHE[key]


def kernel(h_u_cross, h_u_target, dom_movie, w_q, w_k, w_v, b_q, b_k, b_v,
           w_o, b_o, ln1_g, ln1_b, w1, b1, w2, b2, ln2_g, ln2_b,
           trace=False, r_blk=512, **run_kwargs):
    inputs = dict(dom_movie=dom_movie, w_q=w_q, w_k=w_k, w_v=w_v, b_q=b_q,
                  b_k=b_k, b_v=b_v, w_o=w_o, b_o=b_o, ln1_g=ln1_g,
                  ln1_b=ln1_b, w1=w1, b1=b1, w2=w2, b2=b2, ln2_g=ln2_g,
                  ln2_b=ln2_b)
    prep, scales, t1, t2 = host_prepare(inputs)
    nc = _get_program(B_CORE, r_blk, scales, t1, t2)

    xc = np.asarray(h_u_cross, np.float32)
    xt = np.asarray(h_u_target, np.float32)
    s = (0.5 * (xc + xt)).astype(np.float32)
    d = (xc - xt).astype(np.float32)
    S1 = s.astype(E4)
    S2 = (s - S1.astype(np.float32)).astype(E4)
    D8 = d.astype(E4)
    s1m = _fm_acts(S1)
    s2m = _fm_acts(S2)
    d8m = _fm_acts(D8)

    in_maps = []
    for c in range(N_CORES):
        m = dict(prep)
        sl = slice(c * B_CORE, (c + 1) * B_CORE)
        m["s1"] = np.ascontiguousarray(s1m[:, :, sl])
        m["s2"] = np.ascontiguousarray(s2m[:, :, sl])
        m["d8"] = np.ascontiguousarray(d8m[:, :, sl])
        in_maps.append(m)

    res = run_bass_kernel_spmd(nc, in_maps, core_ids=list(range(N_CORES)),
                               trace=trace, **run_kwargs)
    outs = []
    for c in range(N_CORES):
        o = res.results[c]["out"]  # [128, MT, B_CORE]
        outs.append(o.transpose(2, 1, 0).reshape(B_CORE, D))
    kernel.last_results = res
    return np.ascontiguousarray(np.concatenate(outs, axis=0), np.float32)
